# revision 1
# baseline (speedup 1.0000x reference)
"""MoE layer (shared expert + 8 routed experts, top-2 sigmoid router) on 8
Trainium2 NeuronCores.

Strategy: data-parallel over tokens. N = 4*2048 = 8192 tokens split into 8
shards of 1024. Each core computes the full layer for its tokens:
  - router (fp32 PE matmuls; exact top-2 via DVE max8 + match_replace)
  - dense all-expert MLPs in fp32r (shared + 8 routed), with the per-token
    combine weight folded in as sqrt(c) before the squared-relu:
       relu(x @ w1)^2 * c == (relu(x @ w1) * sqrt(c))^2
    so the routed outputs accumulate with no post-scaling.

Activations live transposed on-chip ([C, tokens]; C on partitions), so both
MLP matmuls use the weights exactly as stored ([in, out]) as the stationary
operand and no activation transposes are needed.
"""
import sys
import types

sys.path.insert(0, '/opt/trn_rl_repo')

import numpy as np

import concourse.bass as bass
import concourse.mybir as mybir
import concourse.tile as tile
from concourse import bacc
from concourse.bass_utils import run_bass_kernel_spmd
from concourse.masks import make_identity

f32 = mybir.dt.float32
f32r = mybir.dt.float32r
AF = mybir.ActivationFunctionType
ALU = mybir.AluOpType

N_CORES = 8
B, T, C = 4, 2048, 768
E, K = 8, 2
N_TOK = B * T
TLOC = N_TOK // N_CORES       # tokens per core (1024)
KT = C // 128                 # 6 contraction tiles
TB = TLOC // 128              # 8 token blocks (router)
TH = TLOC // 512              # 2 moving-dim chunks of 512
NEXP = E + 1                  # shared expert runs as expert 0


def _build():
    nc = bacc.Bacc("TRN2", target_bir_lowering=False, debug=False,
                   num_devices=N_CORES)

    x_T = nc.declare_dram_parameter("x_T", [C, TLOC], f32, isOutput=False)
    x_Tr = nc.declare_dram_parameter("x_Tr", [C, TLOC], f32r, isOutput=False)
    rwT = nc.declare_dram_parameter("rwT", [C, E], f32, isOutput=False)
    w1 = nc.declare_dram_parameter("w1", [E, C, C], f32r, isOutput=False)
    w2 = nc.declare_dram_parameter("w2", [E, C, C], f32r, isOutput=False)
    wfc = nc.declare_dram_parameter("wfc", [C, C], f32r, isOutput=False)
    wproj = nc.declare_dram_parameter("wproj", [C, C], f32r, isOutput=False)
    o_yT = nc.declare_dram_parameter("o_yT", [C, TLOC], f32, isOutput=True)
    o_comb = nc.declare_dram_parameter("o_comb", [TB, 128, E], f32, isOutput=True)

    sqcT_dram = nc.dram_tensor("sqcT_dram", [E, TLOC], f32)

    with tile.TileContext(nc) as tc:
        with (
            tc.tile_pool(name="const", bufs=1) as cpool,
            tc.tile_pool(name="acts", bufs=1) as apool,
            tc.tile_pool(name="wts", bufs=2) as wpool,
            tc.tile_pool(name="small", bufs=2) as spool,
            tc.tile_pool(name="tbuf", bufs=2) as tpool,
            tc.tile_pool(name="bcast", bufs=2) as bpool,
            tc.tile_pool(name="ps_h", bufs=2, space="PSUM") as ps_h_pool,
            tc.tile_pool(name="ps_y", bufs=2, space="PSUM") as ps_y_pool,
        ):
            ident = cpool.tile([128, 128], f32)
            make_identity(nc, ident[:])

            rwt = cpool.tile([128, KT, E], f32)
            nc.sync.dma_start(rwt[:], rwT.rearrange("(k p) e -> p k e", p=128))
            xt = []
            xtr = []
            for k in range(KT):
                xt_k = apool.tile([128, TLOC], f32, tag=f"xt{k}")
                nc.sync.dma_start(xt_k[:], x_T[k * 128:(k + 1) * 128, :])
                xt.append(xt_k)
            for k in range(KT):
                xtr_k = apool.tile([128, TLOC], f32r, tag=f"xtr{k}")
                nc.sync.dma_start(xtr_k[:], x_Tr[k * 128:(k + 1) * 128, :])
                xtr.append(xtr_k)

            # ---------------- router ----------------
            sqcT = apool.tile([E, TLOC], f32)
            for tb in range(TB):
                blk = slice(tb * 128, (tb + 1) * 128)
                ps_l = ps_h_pool.tile([128, E], f32, tag="psh0")
                for k in range(KT):
                    nc.tensor.matmul(ps_l[:], xt[k][:, blk], rwt[:, k, :],
                                     start=(k == 0), stop=(k == KT - 1))
                scores = spool.tile([128, E], f32, tag="scores")
                nc.scalar.activation(scores[:], ps_l[:], AF.Sigmoid)
                top8 = spool.tile([128, E], f32, tag="top8")
                nc.vector.max(top8[:], scores[:])
                mr = spool.tile([128, E], f32, tag="mr")
                nc.vector.tensor_copy(mr[:, 0:K], top8[:, 0:K])
                nc.vector.memset(mr[:, K:], 0.0)
                zap = spool.tile([128, E], f32, tag="zap")
                nc.vector.match_replace(zap[:], mr[:], scores[:], 0.0)
                msk = spool.tile([128, E], f32, tag="msk")
                nc.vector.tensor_sub(msk[:], scores[:], zap[:])
                den = spool.tile([128, 1], f32, tag="den")
                nc.vector.reduce_sum(den[:], msk[:], mybir.AxisListType.X)
                rden = spool.tile([128, 1], f32, tag="rden")
                nc.vector.reciprocal(rden[:], den[:])
                comb = spool.tile([128, E], f32, tag="comb")
                nc.vector.tensor_scalar_mul(comb[:], msk[:], rden[:])
                nc.sync.dma_start(o_comb[tb], comb[:])
                sqc = spool.tile([128, E], f32, tag="sqc")
                nc.scalar.activation(sqc[:], comb[:], AF.Sqrt)
                ps_t = ps_h_pool.tile([E, 128], f32, tag="psh1")
                nc.tensor.transpose(ps_t[:], sqc[:], ident[:])
                nc.scalar.activation(sqcT[:, blk], ps_t[:], AF.Copy)
            nc.sync.dma_start(sqcT_dram[:], sqcT[:])

            # ---------------- experts ----------------
            yacc = apool.tile([128, KT, TLOC], f32)
            hsq = apool.tile([128, KT, TLOC], f32r)

            for ei in range(NEXP):
                routed = ei > 0
                e = ei - 1
                if routed:
                    w1_src = w1[e].rearrange("(k p) m -> p k m", p=128)
                    w2_src = w2[e].rearrange("(k p) m -> p k m", p=128)
                else:
                    w1_src = wfc.rearrange("(k p) m -> p k m", p=128)
                    w2_src = wproj.rearrange("(k p) m -> p k m", p=128)
                w1sb = wpool.tile([128, KT, C], f32r, tag="w1")
                w2sb = wpool.tile([128, KT, C], f32r, tag="w2")
                for k in range(KT):
                    nc.sync.dma_start(w1sb[:, k, :], w1_src[:, k, :])
                    nc.sync.dma_start(w2sb[:, k, :], w2_src[:, k, :])
                if routed:
                    bca = bpool.tile([128, TLOC], f32, tag="bca")
                    nc.sync.dma_start(
                        bca[:], sqcT_dram[e:e + 1, :].to_broadcast([128, TLOC]))

                # layer 1: hsq[ho] = (relu(w1[:,ho].T @ xT) * sqrt(c))^2
                # k outer / th inner keeps the two 512-token matmuls of each
                # weight tile back-to-back so the stationary operand is reused.
                for ho in range(KT):
                    mo = slice(ho * 128, (ho + 1) * 128)
                    psh0 = ps_h_pool.tile([128, 512], f32, tag="psh0")
                    psh1 = ps_h_pool.tile([128, 512], f32, tag="psh1")
                    psh = [psh0, psh1]
                    for k in range(KT):
                        for th in range(TH):
                            ts = slice(th * 512, (th + 1) * 512)
                            nc.tensor.matmul(psh[th][:], w1sb[:, k, mo],
                                             xtr[k][:, ts],
                                             start=(k == 0), stop=(k == KT - 1))
                    for th in range(TH):
                        ts = slice(th * 512, (th + 1) * 512)
                        t_ = tpool.tile([128, 512], f32, tag=f"t{th}")
                        if routed:
                            nc.vector.scalar_tensor_tensor(
                                t_[:], psh[th][:], 0.0, bca[:, ts],
                                op0=ALU.max, op1=ALU.mult)
                        else:
                            nc.vector.tensor_scalar_max(t_[:], psh[th][:], 0.0)
                        nc.scalar.activation(hsq[:, ho, ts], t_[:], AF.Square)

                # layer 2: yacc += w2[:,co].T @ hsq
                for co in range(KT):
                    mo = slice(co * 128, (co + 1) * 128)
                    psy0 = ps_y_pool.tile([128, 512], f32, tag="psy0")
                    psy1 = ps_y_pool.tile([128, 512], f32, tag="psy1")
                    psy = [psy0, psy1]
                    for k in range(KT):
                        for th in range(TH):
                            ts = slice(th * 512, (th + 1) * 512)
                            nc.tensor.matmul(psy[th][:], w2sb[:, k, mo],
                                             hsq[:, k, ts],
                                             start=(k == 0), stop=(k == KT - 1))
                    for th in range(TH):
                        ts = slice(th * 512, (th + 1) * 512)
                        if ei == 0:
                            nc.vector.tensor_copy(yacc[:, co, ts], psy[th][:])
                        else:
                            nc.vector.tensor_add(yacc[:, co, ts],
                                                 yacc[:, co, ts], psy[th][:])

            for k in range(KT):
                nc.sync.dma_start(o_yT[k * 128:(k + 1) * 128, :], yacc[:, k, :])
    nc.compile()
    return nc


_NC_CACHE = None


def _get_nc():
    global _NC_CACHE
    if _NC_CACHE is None:
        _NC_CACHE = _build()
    return _NC_CACHE


def kernel(x, w_fc_sh, w_proj_sh, w1, w2, router_w, balance_bias):
    x = np.ascontiguousarray(np.asarray(x, np.float32))
    w1 = np.ascontiguousarray(np.asarray(w1, np.float32))
    w2 = np.ascontiguousarray(np.asarray(w2, np.float32))
    wfc = np.ascontiguousarray(np.asarray(w_fc_sh, np.float32))
    wproj = np.ascontiguousarray(np.asarray(w_proj_sh, np.float32))
    rwT = np.ascontiguousarray(np.asarray(router_w, np.float32).T)

    nc = _get_nc()

    xf = x.reshape(N_TOK, C)
    in_maps = []
    for i in range(N_CORES):
        xT = np.ascontiguousarray(xf[i * TLOC:(i + 1) * TLOC].T)
        in_maps.append({
            "x_T": xT, "x_Tr": xT, "rwT": rwT,
            "w1": w1, "w2": w2, "wfc": wfc, "wproj": wproj,
        })

    res = run_bass_kernel_spmd(nc, in_maps, list(range(N_CORES)))
    shards = [res.results[i]["o_yT"].T for i in range(N_CORES)]
    out = np.concatenate(shards, axis=0).reshape(B, T, C).astype(np.float32)
    kernel._last_results = res
    return out



# revision 6
# speedup vs baseline: 1.0506x; 1.0506x over previous
"""MoE layer (shared expert + 8 routed experts, top-2 sigmoid router) on 8
Trainium2 NeuronCores — sparse dispatch version.

Strategy: data-parallel over tokens (1024/core). Each core:
  1. Router in exact fp32 on PE (top-2 via DVE max8 + match_replace).
  2. Sparse dispatch: instead of dense all-expert compute, each token is
     dispatched to only its top-2 experts via SWDGE dma_gather into a
     per-expert capacity buffer (CAP=384 slots/expert; E[count]=256).
     The per-token gate weight tw is folded in as x*sqrt(tw) before
     dispatch, since relu(a*x@w1)^2 @ w2 = a^2 * (relu(x@w1)^2 @ w2).
  3. The slot->token index map is built ON DEVICE: positions via an
     exclusive cumsum (triangular-matrix matmuls), then inverted with a
     dma_scatter_add of per-token metadata rows into a slot-indexed DRAM
     array, and read back as int16 gather/scatter index lists.
  4. Routed experts run in bf16 (PE rate identical to fp32r, half the
     weight DMA); layer 2 is computed slot-major (stationary = h^T tile)
     so the output rows are slots, scatter-added straight into the
     token-major output which was pre-filled with the shared-expert MLP.
"""
import sys

sys.path.insert(0, '/opt/trn_rl_repo')

import numpy as np
import ml_dtypes

import concourse.bass as bass
import concourse.mybir as mybir
import concourse.tile as tile
from concourse import bacc
from concourse.bass_utils import run_bass_kernel_spmd

f32 = mybir.dt.float32
bf16 = mybir.dt.bfloat16
i16 = mybir.dt.int16
AF = mybir.ActivationFunctionType
ALU = mybir.AluOpType

N_CORES = 8
B, T, C = 4, 2048, 768
E, K = 8, 2
N_TOK = B * T
TLOC = N_TOK // N_CORES      # tokens per core (1024)
KT = C // 128                # 6 contraction tiles
TB = TLOC // 128             # 8 token blocks
CAP = 384                    # per-expert slot capacity (mean count = 256)
S = E * CAP                  # 3072 total slots
SB = S // 128                # 24 slot blocks
ECOLS = CAP // 16            # idx columns per expert (24)
EB = CAP // 128              # slot blocks per expert (3)
XS_ROWS = 2 * TLOC + 128     # dispatch buffer rows (row 2048 = zeros)
ZROW = 2 * TLOC              # zero row index
OUT_ROWS = TLOC + 128        # output rows (row 1024 = pad-slot dummy)
DUMMY = TLOC


def _build():
    nc = bacc.Bacc("TRN2", target_bir_lowering=False, debug=False,
                   num_devices=N_CORES)

    x_T = nc.declare_dram_parameter("x_T", [C, TLOC], f32, isOutput=False)
    x_T16 = nc.declare_dram_parameter("x_T16", [C, TLOC], bf16, isOutput=False)
    x_tm = nc.declare_dram_parameter("x_tm", [TLOC, C], f32, isOutput=False)
    rwT = nc.declare_dram_parameter("rwT", [C, E], f32, isOutput=False)
    w1 = nc.declare_dram_parameter("w1", [E, C, C], bf16, isOutput=False)
    w2 = nc.declare_dram_parameter("w2", [E, C, C], bf16, isOutput=False)
    wfc = nc.declare_dram_parameter("wfc", [C, C], bf16, isOutput=False)
    wproj = nc.declare_dram_parameter("wproj", [C, C], bf16, isOutput=False)
    iotaP = nc.declare_dram_parameter("iotaP", [128, 1], f32, isOutput=False)
    trid = nc.declare_dram_parameter("trid", [128, 128], f32, isOutput=False)
    iota8 = nc.declare_dram_parameter("iota8", [1, E], f32, isOutput=False)
    o_out = nc.declare_dram_parameter("o_out", [OUT_ROWS, C], f32,
                                      isOutput=True)
    o_dbg = nc.declare_dram_parameter("o_dbg", [128, 64], f32, isOutput=True)

    xs_dram = nc.dram_tensor("xs_dram", [XS_ROWS, C], bf16)
    meta_dram = nc.dram_tensor("meta_dram", [S, 64], f32)

    with tile.TileContext(nc) as tc:
        with (
            tc.tile_pool(name="const", bufs=1) as cpool,
            tc.tile_pool(name="acts", bufs=1) as apool,
            tc.tile_pool(name="wts", bufs=2) as wpool,
            tc.tile_pool(name="rt", bufs=2) as rpool,
            tc.tile_pool(name="masks", bufs=1) as mpool,
            tc.tile_pool(name="idx", bufs=1) as ipool,
            tc.tile_pool(name="xg", bufs=2) as gpool,
            tc.tile_pool(name="hsq", bufs=2) as hpool,
            tc.tile_pool(name="ysb", bufs=2) as ypool,
            tc.tile_pool(name="ysh", bufs=2) as yshpool,
            tc.tile_pool(name="xsc", bufs=2) as xspool,
            tc.tile_pool(name="ps_small", bufs=2, space="PSUM") as ps_s,
            tc.tile_pool(name="ps_l1", bufs=2, space="PSUM") as ps_l1,
            tc.tile_pool(name="ps_l2", bufs=2, space="PSUM") as ps_l2,
        ):
            # ---------------- constants / inputs ----------------
            tri_sb = cpool.tile([128, 128], f32)
            nc.sync.dma_start(tri_sb[:], trid[:])
            ones_sb = cpool.tile([128, 128], f32)
            nc.vector.memset(ones_sb[:], 1.0)
            iotaP_sb = cpool.tile([128, 1], f32)
            nc.sync.dma_start(iotaP_sb[:], iotaP[:])
            e384_sb = cpool.tile([128, E], f32)
            nc.sync.dma_start(e384_sb[:], iota8[0:1, :].to_broadcast([128, E]))
            nc.vector.tensor_scalar_mul(e384_sb[:], e384_sb[:], float(CAP))
            rwt = cpool.tile([128, KT, E], f32)
            nc.sync.dma_start(rwt[:], rwT.rearrange("(k p) e -> p k e", p=128))

            xt = []
            for k in range(KT):
                xt_k = apool.tile([128, TLOC], f32, tag=f"xt{k}")
                nc.sync.dma_start(xt_k[:], x_T[k * 128:(k + 1) * 128, :])
                xt.append(xt_k)
            x16 = apool.tile([128, KT, TLOC], bf16)
            nc.sync.dma_start(x16[:], x_T16.rearrange("(k p) n -> p k n", p=128))
            xtm = apool.tile([128, TB, C], f32)
            nc.sync.dma_start(xtm[:], x_tm.rearrange("(b p) c -> p b c", p=128))

            # shared-expert weights go through the same rotating pool slots
            # as the routed experts (they are "expert -1")
            wfc_sb = wpool.tile([128, KT, C], bf16, tag="w1")
            wproj_sb = wpool.tile([128, KT, C], bf16, tag="w2")
            for k in range(KT):
                nc.sync.dma_start(
                    wfc_sb[:, k, :],
                    wfc.rearrange("(k p) m -> p k m", p=128)[:, k, :])
                nc.sync.dma_start(
                    wproj_sb[:, k, :],
                    wproj.rearrange("(k p) m -> p k m", p=128)[:, k, :])

            # index-construction tiles
            slots_tm = ipool.tile([128, 2 * TB], f32)     # col b = k*8+tb
            content = ipool.tile([128, 2 * TB, 64], f32)  # scatter payload
            nc.vector.memset(content[:], 0.0)

            # ---------------- router ----------------
            m_list, m1_list, m2_list = [], [], []
            for tb in range(TB):
                blk = slice(tb * 128, (tb + 1) * 128)
                ps_r = ps_s.tile([128, E], f32, tag="small")
                for k in range(KT):
                    nc.tensor.matmul(ps_r[:], xt[k][:, blk], rwt[:, k, :],
                                     start=(k == 0), stop=(k == KT - 1))
                scores = rpool.tile([128, E], f32, tag="scores")
                nc.scalar.activation(scores[:], ps_r[:], AF.Sigmoid)
                top8 = rpool.tile([128, E], f32, tag="top8")
                nc.vector.max(top8[:], scores[:])
                mr = rpool.tile([128, E], f32, tag="mr")
                nc.vector.tensor_copy(mr[:, 0:K], top8[:, 0:K])
                nc.vector.memset(mr[:, K:], 0.0)
                zap = rpool.tile([128, E], f32, tag="zap")
                nc.vector.match_replace(zap[:], mr[:], scores[:], 0.0)
                msk = rpool.tile([128, E], f32, tag="msk")
                nc.vector.tensor_sub(msk[:], scores[:], zap[:])
                den = rpool.tile([128, 1], f32, tag="den")
                nc.vector.reduce_sum(den[:], msk[:], mybir.AxisListType.X)
                rden = rpool.tile([128, 1], f32, tag="rden")
                nc.vector.reciprocal(rden[:], den[:])
                twv = rpool.tile([128, K], f32, tag="twv")
                nc.vector.tensor_scalar_mul(twv[:], top8[:, 0:K], rden[:])
                sqtw = rpool.tile([128, K], f32, tag="sqtw")
                nc.scalar.activation(sqtw[:], twv[:], AF.Sqrt)

                m_t = mpool.tile([128, E], f32, tag=f"m{tb}")
                nc.vector.tensor_scalar(m_t[:], msk[:], 0.0, None, op0=ALU.is_gt)
                m1_t = mpool.tile([128, E], f32, tag=f"m1{tb}")
                nc.vector.tensor_scalar(m1_t[:], msk[:], top8[:, 0:1], None,
                                        op0=ALU.is_equal)
                m2_t = mpool.tile([128, E], f32, tag=f"m2{tb}")
                nc.vector.tensor_sub(m2_t[:], m_t[:], m1_t[:])
                m_list.append(m_t)
                m1_list.append(m1_t)
                m2_list.append(m2_t)

                # scaled dispatch copies: xs[k*1024 + n] = x[n] * sqrt(tw_k)
                for k in range(K):
                    xsk = xspool.tile([128, C], bf16, tag=f"xs{k}")
                    nc.vector.tensor_scalar_mul(xsk[:], xtm[:, tb, :],
                                                sqtw[:, k:k + 1])
                    nc.sync.dma_start(
                        xs_dram[k * TLOC + tb * 128:k * TLOC + (tb + 1) * 128, :],
                        xsk[:])

            zrow16 = ipool.tile([128, C], bf16)
            nc.vector.memset(zrow16[:], 0.0)
            nc.sync.dma_start(xs_dram[ZROW:ZROW + 128, :], zrow16[:])

            # ---------------- positions via exclusive cumsum ----------------
            for tb in range(TB):
                ps_c = ps_s.tile([128, E], f32, tag="small")
                for tb2 in range(tb):
                    nc.tensor.matmul(ps_c[:], ones_sb[:], m_list[tb2][:],
                                     start=(tb2 == 0), stop=False)
                nc.tensor.matmul(ps_c[:], tri_sb[:], m_list[tb][:],
                                 start=(tb == 0), stop=True)
                posc = rpool.tile([128, E], f32, tag="posc")
                nc.vector.tensor_scalar_min(posc[:], ps_c[:], float(CAP - 1))
                sl = rpool.tile([128, E], f32, tag="sl")
                nc.vector.tensor_add(sl[:], posc[:], e384_sb[:])
                t1 = rpool.tile([128, E], f32, tag="t1")
                nc.vector.tensor_mul(t1[:], sl[:], m1_list[tb][:])
                nc.vector.reduce_sum(slots_tm[:, tb:tb + 1], t1[:],
                                     mybir.AxisListType.X)
                t2 = rpool.tile([128, E], f32, tag="t2")
                nc.vector.tensor_mul(t2[:], sl[:], m2_list[tb][:])
                nc.vector.reduce_sum(slots_tm[:, TB + tb:TB + tb + 1], t2[:],
                                     mybir.AxisListType.X)
                # scatter payload: col0 = gather row (k*1024+n-2048),
                # col1 = scatter row (n-1024)
                nc.vector.tensor_scalar_add(content[:, tb, 0:1], iotaP_sb[:],
                                            float(tb * 128 - 2 * TLOC))
                nc.vector.tensor_scalar_add(content[:, TB + tb, 0:1],
                                            iotaP_sb[:],
                                            float(TLOC + tb * 128 - 2 * TLOC))
                nc.vector.tensor_scalar_add(content[:, tb, 1:2], iotaP_sb[:],
                                            float(tb * 128 - TLOC))
                nc.vector.tensor_scalar_add(content[:, TB + tb, 1:2],
                                            iotaP_sb[:],
                                            float(tb * 128 - TLOC))

            # ---------------- invert the slot map via scatter-add ----------
            slots_i16 = ipool.tile([128, 2 * TB], i16)
            nc.vector.tensor_copy(slots_i16[:], slots_tm[:])
            inv_idx = ipool.tile([128, 2 * TLOC // 16], i16)  # [128, 128]
            inv_r = inv_idx[:].rearrange("p (c r) -> p c r", r=8)
            for r in range(8):
                nc.sync.dma_start(inv_r[0:16, :, r],
                                  slots_i16[r * 16:(r + 1) * 16, :])
            for rr in range(1, 8):
                nc.sync.dma_start(inv_idx[rr * 16:(rr + 1) * 16, :],
                                  inv_idx[0:16, :])

            zmeta = ipool.tile([128, SB, 64], f32)
            nc.vector.memset(zmeta[:], 0.0)
            nc.sync.dma_start(meta_dram.rearrange("(b p) c -> p b c", p=128),
                              zmeta[:])
            nc.gpsimd.dma_scatter_add(meta_dram[:], content[:], inv_idx[:],
                                      2 * TLOC, 2 * TLOC, 64)
            meta_sb = ipool.tile([128, SB, 64], f32)
            nc.sync.dma_start(meta_sb[:],
                              meta_dram.rearrange("(b p) c -> p b c", p=128))

            gidx_f = ipool.tile([128, SB], f32)
            nc.vector.tensor_scalar(gidx_f[:], meta_sb[:, :, 0], float(ZROW),
                                    float(ZROW), op0=ALU.add, op1=ALU.min)
            nc.vector.tensor_scalar_max(gidx_f[:], gidx_f[:], 0.0)
            sidx_f = ipool.tile([128, SB], f32)
            nc.vector.tensor_scalar(sidx_f[:], meta_sb[:, :, 1], float(DUMMY),
                                    float(DUMMY), op0=ALU.add, op1=ALU.min)
            nc.vector.tensor_scalar_max(sidx_f[:], sidx_f[:], 0.0)

            gidx16 = ipool.tile([128, S // 16], i16)
            sidx16 = ipool.tile([128, S // 16], i16)
            g16t = ipool.tile([128, SB], i16)
            s16t = ipool.tile([128, SB], i16)
            nc.vector.tensor_copy(g16t[:], gidx_f[:])
            nc.vector.tensor_copy(s16t[:], sidx_f[:])
            g16r = gidx16[:].rearrange("p (c r) -> p c r", r=8)
            s16r = sidx16[:].rearrange("p (c r) -> p c r", r=8)
            for r in range(8):
                nc.sync.dma_start(g16r[0:16, :, r],
                                  g16t[r * 16:(r + 1) * 16, :])
                nc.sync.dma_start(s16r[0:16, :, r],
                                  s16t[r * 16:(r + 1) * 16, :])
            for rr in range(1, 8):
                nc.sync.dma_start(gidx16[rr * 16:(rr + 1) * 16, :],
                                  gidx16[0:16, :])
                nc.sync.dma_start(sidx16[rr * 16:(rr + 1) * 16, :],
                                  sidx16[0:16, :])

            # debug: slots + final idx values
            nc.sync.dma_start(o_dbg[:, 0:16], slots_tm[:])
            nc.sync.dma_start(o_dbg[:, 16:40], gidx_f[:])
            nc.sync.dma_start(o_dbg[:, 40:64], sidx_f[:])

            # ---------------- shared expert (bf16) ----------------
            h_sh = apool.tile([128, KT, TLOC], bf16)
            for ho in range(KT):
                mo = slice(ho * 128, (ho + 1) * 128)
                for th in range(2):
                    ts = slice(th * 512, (th + 1) * 512)
                    ps = ps_l1.tile([128, 512], f32, tag="l1")
                    for k in range(KT):
                        nc.tensor.matmul(ps[:], wfc_sb[:, k, mo],
                                         x16[:, k, ts],
                                         start=(k == 0), stop=(k == KT - 1))
                    rl = yshpool.tile([128, 512], f32, tag="rl")
                    nc.vector.tensor_scalar_max(rl[:], ps[:], 0.0)
                    nc.scalar.activation(h_sh[:, ho, ts], rl[:], AF.Square)
            for tcb in range(TB):
                tcs = slice(tcb * 128, (tcb + 1) * 128)
                ysh_t = yshpool.tile([128, C], f32, tag="ysh")
                for half in range(2):
                    hs = slice(half * 384, (half + 1) * 384)
                    psx = ps_l2.tile([128, 384], f32, tag=f"l2{half}")
                    for hk in range(KT):
                        nc.tensor.matmul(psx[:], h_sh[:, hk, tcs],
                                         wproj_sb[:, hk, hs],
                                         start=(hk == 0), stop=(hk == KT - 1))
                    nc.scalar.activation(ysh_t[:, hs], psx[:], AF.Copy)
                nc.sync.dma_start(o_out[tcb * 128:(tcb + 1) * 128, :], ysh_t[:])
            zrow32 = ipool.tile([128, C], f32)
            nc.vector.memset(zrow32[:], 0.0)
            nc.sync.dma_start(o_out[DUMMY:DUMMY + 128, :], zrow32[:])

            # ---------------- routed experts ----------------
            def emit_gather(e):
                xg = gpool.tile([128, KT, CAP], bf16, tag="xg")
                nc.gpsimd.dma_gather(xg[:], xs_dram[:],
                                     gidx16[:, ECOLS * e:ECOLS * (e + 1)],
                                     CAP, CAP, C, transpose=True)
                return xg

            xg_cur = emit_gather(0)
            for e in range(E):
                w1sb = wpool.tile([128, KT, C], bf16, tag="w1")
                w2sb = wpool.tile([128, KT, C], bf16, tag="w2")
                w1_src = w1[e].rearrange("(k p) m -> p k m", p=128)
                w2_src = w2[e].rearrange("(k p) m -> p k m", p=128)
                for k in range(KT):
                    nc.sync.dma_start(w1sb[:, k, :], w1_src[:, k, :])
                    nc.sync.dma_start(w2sb[:, k, :], w2_src[:, k, :])

                xg_next = emit_gather(e + 1) if e + 1 < E else None

                hsq = hpool.tile([128, KT, CAP], bf16, tag="hsq")
                for ho in range(KT):
                    mo = slice(ho * 128, (ho + 1) * 128)
                    ps = ps_l1.tile([128, 512], f32, tag="l1")
                    for k in range(KT):
                        nc.tensor.matmul(ps[:, 0:CAP], w1sb[:, k, mo],
                                         xg_cur[:, k, :],
                                         start=(k == 0), stop=(k == KT - 1))
                    rl = yshpool.tile([128, 512], f32, tag="rl")
                    nc.vector.tensor_scalar_max(rl[:, 0:CAP], ps[:, 0:CAP], 0.0)
                    nc.scalar.activation(hsq[:, ho, :], rl[:, 0:CAP], AF.Square)

                ysb = ypool.tile([128, EB, C], f32, tag="ysb")
                for sc in range(EB):
                    scs = slice(sc * 128, (sc + 1) * 128)
                    for half in range(2):
                        hs = slice(half * 384, (half + 1) * 384)
                        psx = ps_l2.tile([128, 384], f32, tag=f"l2{half}")
                        for hk in range(KT):
                            nc.tensor.matmul(psx[:], hsq[:, hk, scs],
                                             w2sb[:, hk, hs],
                                             start=(hk == 0),
                                             stop=(hk == KT - 1))
                        nc.scalar.activation(ysb[:, sc, hs], psx[:], AF.Copy)
                nc.gpsimd.dma_scatter_add(o_out[:], ysb[:],
                                          sidx16[:, ECOLS * e:ECOLS * (e + 1)],
                                          CAP, CAP, C)
                xg_cur = xg_next
    nc.compile()
    return nc


_NC_CACHE = None


def _get_nc():
    global _NC_CACHE
    if _NC_CACHE is None:
        _NC_CACHE = _build()
    return _NC_CACHE


def make_in_maps(x, w_fc_sh, w_proj_sh, w1, w2, router_w):
    x = np.ascontiguousarray(np.asarray(x, np.float32))
    bfl = ml_dtypes.bfloat16
    w1b = np.ascontiguousarray(np.asarray(w1, np.float32).astype(bfl))
    w2b = np.ascontiguousarray(np.asarray(w2, np.float32).astype(bfl))
    wfcb = np.ascontiguousarray(np.asarray(w_fc_sh, np.float32).astype(bfl))
    wprojb = np.ascontiguousarray(np.asarray(w_proj_sh, np.float32).astype(bfl))
    rwT = np.ascontiguousarray(np.asarray(router_w, np.float32).T)
    iotaP = np.arange(128, dtype=np.float32).reshape(128, 1)
    trid = np.triu(np.ones((128, 128), np.float32), 1)
    iota8 = np.arange(E, dtype=np.float32).reshape(1, E)

    xf = x.reshape(N_TOK, C)
    in_maps = []
    for i in range(N_CORES):
        xs = xf[i * TLOC:(i + 1) * TLOC]
        xT = np.ascontiguousarray(xs.T)
        in_maps.append({
            "x_T": xT,
            "x_T16": np.ascontiguousarray(xT.astype(bfl)),
            "x_tm": np.ascontiguousarray(xs),
            "rwT": rwT, "w1": w1b, "w2": w2b,
            "wfc": wfcb, "wproj": wprojb,
            "iotaP": iotaP, "trid": trid, "iota8": iota8,
        })
    return in_maps


def kernel(x, w_fc_sh, w_proj_sh, w1, w2, router_w, balance_bias):
    nc = _get_nc()
    in_maps = make_in_maps(x, w_fc_sh, w_proj_sh, w1, w2, router_w)
    res = run_bass_kernel_spmd(nc, in_maps, list(range(N_CORES)))
    shards = [np.asarray(res.results[i]["o_out"])[:TLOC]
              for i in range(N_CORES)]
    out = np.concatenate(shards, axis=0).reshape(B, T, C).astype(np.float32)
    kernel._last_results = res
    return out


# revision 8
# speedup vs baseline: 1.1960x; 1.1384x over previous
"""MoE layer (shared expert + 8 routed experts, top-2 sigmoid router) on 8
Trainium2 NeuronCores — sparse dispatch version.

Strategy: data-parallel over tokens (1024/core). Each core:
  1. Router in exact fp32 on PE (top-2 via DVE max8 + match_replace).
  2. Sparse dispatch: each token goes to only its top-2 experts via SWDGE
     dma_gather into a per-expert capacity buffer (CAP=384, E[count]=256).
     The gate weight tw is folded in as x*sqrt(tw) before dispatch since
     relu(a*x@w1)^2 @ w2 = a^2 * (relu(x@w1)^2 @ w2).
  3. The slot->token map is built ON DEVICE: exclusive cumsum over the
     top-2 masks (triangular-matrix matmuls), inverted by scatter-adding
     per-token metadata rows into a slot-indexed DRAM array, read back
     directly in the SWDGE index layout via a strided DMA.
  4. Routed experts run in bf16; layer 2 is slot-major (stationary = h^T)
     so outputs are slot rows, scatter-added onto the token-major output
     pre-filled with the shared-expert MLP.

Emission order is tuned so the dispatch-index critical path (DVE + gpsimd)
completes while the PE runs the shared expert, and small index DMAs are
spread across the scalar/vector/gpsimd queues to avoid head-of-line
blocking behind bulk weight DMAs on the sync queue.
"""
import sys

sys.path.insert(0, '/opt/trn_rl_repo')

import numpy as np
import ml_dtypes

import concourse.bass as bass
import concourse.mybir as mybir
import concourse.tile as tile
from concourse import bacc
from concourse.bass_utils import run_bass_kernel_spmd

f32 = mybir.dt.float32
bf16 = mybir.dt.bfloat16
i16 = mybir.dt.int16
AF = mybir.ActivationFunctionType
ALU = mybir.AluOpType

N_CORES = 8
B, T, C = 4, 2048, 768
E, K = 8, 2
N_TOK = B * T
TLOC = N_TOK // N_CORES      # tokens per core (1024)
KT = C // 128                # 6 contraction tiles
TB = TLOC // 128             # 8 token blocks
CAP = 384                    # per-expert slot capacity (mean count = 256)
S = E * CAP                  # 3072 total slots
SB = S // 128                # 24 slot blocks
ECOLS = CAP // 16            # idx columns per expert (24)
EB = CAP // 128              # slot blocks per expert (3)
XS_ROWS = 2 * TLOC + 128     # dispatch buffer rows (rows 2048.. = zeros)
ZROW = 2 * TLOC              # zero row index
OUT_ROWS = TLOC + 128        # output rows (row 1024 = pad-slot dummy)
DUMMY = TLOC


def _build():
    nc = bacc.Bacc("TRN2", target_bir_lowering=False, debug=False,
                   num_devices=N_CORES)

    x_T = nc.declare_dram_parameter("x_T", [C, TLOC], f32, isOutput=False)
    x_T16 = nc.declare_dram_parameter("x_T16", [C, TLOC], bf16, isOutput=False)
    x_tm = nc.declare_dram_parameter("x_tm", [TLOC, C], f32, isOutput=False)
    rwT = nc.declare_dram_parameter("rwT", [C, E], f32, isOutput=False)
    w1 = nc.declare_dram_parameter("w1", [E, C, C], bf16, isOutput=False)
    w2 = nc.declare_dram_parameter("w2", [E, C, C], bf16, isOutput=False)
    wfc = nc.declare_dram_parameter("wfc", [C, C], bf16, isOutput=False)
    wproj = nc.declare_dram_parameter("wproj", [C, C], bf16, isOutput=False)
    iotaP = nc.declare_dram_parameter("iotaP", [128, 1], f32, isOutput=False)
    trid = nc.declare_dram_parameter("trid", [128, 128], f32, isOutput=False)
    iota8 = nc.declare_dram_parameter("iota8", [1, E], f32, isOutput=False)
    o_out = nc.declare_dram_parameter("o_out", [OUT_ROWS, C], f32,
                                      isOutput=True)
    o_dbg = nc.declare_dram_parameter("o_dbg", [128, 16], f32, isOutput=True)

    xs_dram = nc.dram_tensor("xs_dram", [XS_ROWS, C], bf16)
    meta_dram = nc.dram_tensor("meta_dram", [S, 64], f32)

    with tile.TileContext(nc) as tc:
        with (
            tc.tile_pool(name="const", bufs=1) as cpool,
            tc.tile_pool(name="acts", bufs=1) as apool,
            tc.tile_pool(name="wts", bufs=2) as wpool,
            tc.tile_pool(name="rt", bufs=2) as rpool,
            tc.tile_pool(name="masks", bufs=1) as mpool,
            tc.tile_pool(name="idx", bufs=1) as ipool,
            tc.tile_pool(name="xg", bufs=2) as gpool,
            tc.tile_pool(name="hsq", bufs=2) as hpool,
            tc.tile_pool(name="ysb", bufs=2) as ypool,
            tc.tile_pool(name="ysh", bufs=2) as yshpool,
            tc.tile_pool(name="xsc", bufs=2) as xspool,
            tc.tile_pool(name="ps_small", bufs=2, space="PSUM") as ps_s,
            tc.tile_pool(name="ps_l1", bufs=2, space="PSUM") as ps_l1,
            tc.tile_pool(name="ps_l2", bufs=2, space="PSUM") as ps_l2,
        ):
            # ---------------- constants / inputs ----------------
            tri_sb = cpool.tile([128, 128], f32)
            nc.sync.dma_start(tri_sb[:], trid[:])
            iotaP_sb = cpool.tile([128, 1], f32)
            nc.sync.dma_start(iotaP_sb[:], iotaP[:])
            e384_sb = cpool.tile([128, E], f32)
            nc.sync.dma_start(e384_sb[:], iota8[0:1, :].to_broadcast([128, E]))
            nc.vector.tensor_scalar_mul(e384_sb[:], e384_sb[:], float(CAP))
            rwt = cpool.tile([128, KT, E], f32)
            nc.sync.dma_start(rwt[:], rwT.rearrange("(k p) e -> p k e", p=128))
            ones_sb = cpool.tile([128, 128], f32)
            nc.vector.memset(ones_sb[:], 1.0)

            xt = []
            for k in range(KT):
                xt_k = apool.tile([128, TLOC], f32, tag=f"xt{k}")
                nc.sync.dma_start(xt_k[:], x_T[k * 128:(k + 1) * 128, :])
                xt.append(xt_k)

            # zero the slot-metadata array early (gpsimd queue)
            zmeta = ipool.tile([128, SB, 64], f32)
            nc.vector.memset(zmeta[:], 0.0)
            nc.gpsimd.dma_start(meta_dram.rearrange("(b p) c -> p b c", p=128),
                                zmeta[:])

            x16 = apool.tile([128, KT, TLOC], bf16)
            nc.sync.dma_start(x16[:], x_T16.rearrange("(k p) n -> p k n", p=128))
            xtm = apool.tile([128, TB, C], f32)
            nc.sync.dma_start(xtm[:], x_tm.rearrange("(b p) c -> p b c", p=128))

            wfc_sb = wpool.tile([128, KT, C], bf16, tag="w1")
            wproj_sb = wpool.tile([128, KT, C], bf16, tag="w2")
            for k in range(KT):
                nc.sync.dma_start(
                    wfc_sb[:, k, :],
                    wfc.rearrange("(k p) m -> p k m", p=128)[:, k, :])
                nc.sync.dma_start(
                    wproj_sb[:, k, :],
                    wproj.rearrange("(k p) m -> p k m", p=128)[:, k, :])

            slots_tm = ipool.tile([128, 2 * TB], f32)     # col b = k*8+tb
            content = ipool.tile([128, 2 * TB, 64], f32)  # scatter payload
            nc.vector.memset(content[:], 0.0)

            # ---------------- router ----------------
            m_list, m1_list, m2_list = [], [], []
            sqtw_list = []
            for tb in range(TB):
                blk = slice(tb * 128, (tb + 1) * 128)
                ps_r = ps_s.tile([128, E], f32, tag="small")
                for k in range(KT):
                    nc.tensor.matmul(ps_r[:], xt[k][:, blk], rwt[:, k, :],
                                     start=(k == 0), stop=(k == KT - 1))
                scores = rpool.tile([128, E], f32, tag="scores")
                nc.scalar.activation(scores[:], ps_r[:], AF.Sigmoid)
                top8 = rpool.tile([128, E], f32, tag="top8")
                nc.vector.max(top8[:], scores[:])
                mr = rpool.tile([128, E], f32, tag="mr")
                nc.vector.tensor_copy(mr[:, 0:K], top8[:, 0:K])
                nc.vector.memset(mr[:, K:], 0.0)
                zap = rpool.tile([128, E], f32, tag="zap")
                nc.vector.match_replace(zap[:], mr[:], scores[:], 0.0)
                msk = rpool.tile([128, E], f32, tag="msk")
                nc.vector.tensor_sub(msk[:], scores[:], zap[:])
                den = rpool.tile([128, 1], f32, tag="den")
                nc.vector.reduce_sum(den[:], msk[:], mybir.AxisListType.X)
                rden = rpool.tile([128, 1], f32, tag="rden")
                nc.vector.reciprocal(rden[:], den[:])
                twv = rpool.tile([128, K], f32, tag="twv")
                nc.vector.tensor_scalar_mul(twv[:], top8[:, 0:K], rden[:])
                sqtw = mpool.tile([128, K], f32, tag=f"sq{tb}")
                nc.scalar.activation(sqtw[:], twv[:], AF.Sqrt)
                sqtw_list.append(sqtw)

                m_t = mpool.tile([128, E], f32, tag=f"m{tb}")
                nc.vector.tensor_scalar(m_t[:], msk[:], 0.0, None, op0=ALU.is_gt)
                m1_t = mpool.tile([128, E], f32, tag=f"m1{tb}")
                nc.vector.tensor_scalar(m1_t[:], msk[:], top8[:, 0:1], None,
                                        op0=ALU.is_equal)
                m2_t = mpool.tile([128, E], f32, tag=f"m2{tb}")
                nc.vector.tensor_sub(m2_t[:], m_t[:], m1_t[:])
                m_list.append(m_t)
                m1_list.append(m1_t)
                m2_list.append(m2_t)

            # ---------------- positions via exclusive cumsum ----------------
            for tb in range(TB):
                ps_c = ps_s.tile([128, E], f32, tag="small")
                for tb2 in range(tb):
                    nc.tensor.matmul(ps_c[:], ones_sb[:], m_list[tb2][:],
                                     start=(tb2 == 0), stop=False)
                nc.tensor.matmul(ps_c[:], tri_sb[:], m_list[tb][:],
                                 start=(tb == 0), stop=True)
                posc = rpool.tile([128, E], f32, tag="posc")
                nc.vector.tensor_scalar_min(posc[:], ps_c[:], float(CAP - 1))
                sl = rpool.tile([128, E], f32, tag="sl")
                nc.vector.tensor_add(sl[:], posc[:], e384_sb[:])
                t1 = rpool.tile([128, E], f32, tag="t1")
                nc.vector.tensor_mul(t1[:], sl[:], m1_list[tb][:])
                nc.vector.reduce_sum(slots_tm[:, tb:tb + 1], t1[:],
                                     mybir.AxisListType.X)
                t2 = rpool.tile([128, E], f32, tag="t2")
                nc.vector.tensor_mul(t2[:], sl[:], m2_list[tb][:])
                nc.vector.reduce_sum(slots_tm[:, TB + tb:TB + tb + 1], t2[:],
                                     mybir.AxisListType.X)
                # payload: col0 = gather row - 2048, col1 = scatter row - 1024
                nc.vector.tensor_scalar_add(content[:, tb, 0:1], iotaP_sb[:],
                                            float(tb * 128 - 2 * TLOC))
                nc.vector.tensor_scalar_add(content[:, TB + tb, 0:1],
                                            iotaP_sb[:],
                                            float(TLOC + tb * 128 - 2 * TLOC))
                nc.vector.tensor_scalar_add(content[:, tb, 1:2], iotaP_sb[:],
                                            float(tb * 128 - TLOC))
                nc.vector.tensor_scalar_add(content[:, TB + tb, 1:2],
                                            iotaP_sb[:],
                                            float(tb * 128 - TLOC))

            # ---------------- invert the slot map via scatter-add ----------
            slots_i16 = ipool.tile([128, 2 * TB], i16)
            nc.vector.tensor_copy(slots_i16[:], slots_tm[:])
            inv_idx = ipool.tile([128, 2 * TLOC // 16], i16)  # [128, 128]
            inv_r = inv_idx[:].rearrange("p (c r) -> p c r", r=8)
            for r in range(8):
                eng = nc.scalar if r % 2 == 0 else nc.gpsimd
                eng.dma_start(inv_r[0:16, :, r],
                              slots_i16[r * 16:(r + 1) * 16, :])
            for rr in range(1, 8):
                eng = nc.scalar if rr % 2 == 0 else nc.gpsimd
                eng.dma_start(inv_idx[rr * 16:(rr + 1) * 16, :],
                              inv_idx[0:16, :])

            nc.gpsimd.dma_scatter_add(meta_dram[:], content[:], inv_idx[:],
                                      2 * TLOC, 2 * TLOC, 64)
            # strided readback straight into the SWDGE idx wrap layout:
            # row (b*128 + r*16 + q) -> [q, b*8+r]
            gsb = ipool.tile([16, S // 16, 2], f32)
            meta_r = meta_dram.rearrange("(b r q) c -> q (b r) c", q=16, r=8)
            nc.gpsimd.dma_start(gsb[:], meta_r[:, :, 0:2])

            gidx_f = ipool.tile([16, S // 16], f32)
            nc.vector.tensor_scalar(gidx_f[:], gsb[:, :, 0], float(ZROW),
                                    float(ZROW), op0=ALU.add, op1=ALU.min)
            nc.vector.tensor_scalar_max(gidx_f[:], gidx_f[:], 0.0)
            sidx_f = ipool.tile([16, S // 16], f32)
            nc.vector.tensor_scalar(sidx_f[:], gsb[:, :, 1], float(DUMMY),
                                    float(DUMMY), op0=ALU.add, op1=ALU.min)
            nc.vector.tensor_scalar_max(sidx_f[:], sidx_f[:], 0.0)

            gidx16 = ipool.tile([128, S // 16], i16)
            sidx16 = ipool.tile([128, S // 16], i16)
            nc.vector.tensor_copy(gidx16[0:16, :], gidx_f[:])
            nc.vector.tensor_copy(sidx16[0:16, :], sidx_f[:])
            for rr in range(1, 8):
                nc.scalar.dma_start(gidx16[rr * 16:(rr + 1) * 16, :],
                                    gidx16[0:16, :])
                nc.gpsimd.dma_start(sidx16[rr * 16:(rr + 1) * 16, :],
                                    sidx16[0:16, :])

            # ---------------- scaled dispatch copies -----------------------
            for tb in range(TB):
                for k in range(K):
                    xsk = xspool.tile([128, C], bf16, tag=f"xs{k}")
                    nc.scalar.mul(xsk[:], xtm[:, tb, :],
                                  sqtw_list[tb][:, k:k + 1])
                    nc.sync.dma_start(
                        xs_dram[k * TLOC + tb * 128:k * TLOC + (tb + 1) * 128, :],
                        xsk[:])
            zrow16 = ipool.tile([128, C], bf16)
            nc.vector.memset(zrow16[:], 0.0)
            nc.sync.dma_start(xs_dram[ZROW:ZROW + 128, :], zrow16[:])

            nc.sync.dma_start(o_dbg[:], slots_tm[:])

            # ---------------- shared expert (bf16) ----------------
            h_sh = apool.tile([128, KT, TLOC], bf16)
            for ho in range(KT):
                mo = slice(ho * 128, (ho + 1) * 128)
                for th in range(2):
                    ts = slice(th * 512, (th + 1) * 512)
                    ps = ps_l1.tile([128, 512], f32, tag="l1")
                    for k in range(KT):
                        nc.tensor.matmul(ps[:], wfc_sb[:, k, mo],
                                         x16[:, k, ts],
                                         start=(k == 0), stop=(k == KT - 1))
                    rl = yshpool.tile([128, 512], f32, tag="rl")
                    nc.vector.tensor_scalar_max(rl[:], ps[:], 0.0)
                    nc.scalar.activation(h_sh[:, ho, ts], rl[:], AF.Square)
            for tcb in range(TB):
                tcs = slice(tcb * 128, (tcb + 1) * 128)
                ysh_t = yshpool.tile([128, C], f32, tag="ysh")
                for half in range(2):
                    hs = slice(half * 384, (half + 1) * 384)
                    psx = ps_l2.tile([128, 384], f32, tag=f"l2{half}")
                    for hk in range(KT):
                        nc.tensor.matmul(psx[:], h_sh[:, hk, tcs],
                                         wproj_sb[:, hk, hs],
                                         start=(hk == 0), stop=(hk == KT - 1))
                    nc.scalar.activation(ysh_t[:, hs], psx[:], AF.Copy)
                nc.sync.dma_start(o_out[tcb * 128:(tcb + 1) * 128, :], ysh_t[:])
            zrow32 = ipool.tile([128, C], f32)
            nc.vector.memset(zrow32[:], 0.0)
            nc.sync.dma_start(o_out[DUMMY:DUMMY + 128, :], zrow32[:])

            # ---------------- routed experts ----------------
            def emit_gather(e):
                xg = gpool.tile([128, KT, CAP], bf16, tag="xg")
                nc.gpsimd.dma_gather(xg[:], xs_dram[:],
                                     gidx16[:, ECOLS * e:ECOLS * (e + 1)],
                                     CAP, CAP, C, transpose=True)
                return xg

            xg_cur = emit_gather(0)
            for e in range(E):
                w1sb = wpool.tile([128, KT, C], bf16, tag="w1")
                w2sb = wpool.tile([128, KT, C], bf16, tag="w2")
                w1_src = w1[e].rearrange("(k p) m -> p k m", p=128)
                w2_src = w2[e].rearrange("(k p) m -> p k m", p=128)
                for k in range(KT):
                    nc.sync.dma_start(w1sb[:, k, :], w1_src[:, k, :])
                    nc.sync.dma_start(w2sb[:, k, :], w2_src[:, k, :])

                xg_next = emit_gather(e + 1) if e + 1 < E else None

                hsq = hpool.tile([128, KT, CAP], bf16, tag="hsq")
                for ho in range(KT):
                    mo = slice(ho * 128, (ho + 1) * 128)
                    ps = ps_l1.tile([128, 512], f32, tag="l1")
                    for k in range(KT):
                        nc.tensor.matmul(ps[:, 0:CAP], w1sb[:, k, mo],
                                         xg_cur[:, k, :],
                                         start=(k == 0), stop=(k == KT - 1))
                    rl = yshpool.tile([128, 512], f32, tag="rl")
                    nc.vector.tensor_scalar_max(rl[:, 0:CAP], ps[:, 0:CAP], 0.0)
                    nc.scalar.activation(hsq[:, ho, :], rl[:, 0:CAP], AF.Square)

                ysb = ypool.tile([128, EB, C], f32, tag="ysb")
                for sc in range(EB):
                    scs = slice(sc * 128, (sc + 1) * 128)
                    for half in range(2):
                        hs = slice(half * 384, (half + 1) * 384)
                        psx = ps_l2.tile([128, 384], f32, tag=f"l2{half}")
                        for hk in range(KT):
                            nc.tensor.matmul(psx[:], hsq[:, hk, scs],
                                             w2sb[:, hk, hs],
                                             start=(hk == 0),
                                             stop=(hk == KT - 1))
                        nc.scalar.activation(ysb[:, sc, hs], psx[:], AF.Copy)
                nc.gpsimd.dma_scatter_add(o_out[:], ysb[:],
                                          sidx16[:, ECOLS * e:ECOLS * (e + 1)],
                                          CAP, CAP, C)
                xg_cur = xg_next
    nc.compile()
    return nc


_NC_CACHE = None


def _get_nc():
    global _NC_CACHE
    if _NC_CACHE is None:
        _NC_CACHE = _build()
    return _NC_CACHE


def make_in_maps(x, w_fc_sh, w_proj_sh, w1, w2, router_w):
    x = np.ascontiguousarray(np.asarray(x, np.float32))
    bfl = ml_dtypes.bfloat16
    w1b = np.ascontiguousarray(np.asarray(w1, np.float32).astype(bfl))
    w2b = np.ascontiguousarray(np.asarray(w2, np.float32).astype(bfl))
    wfcb = np.ascontiguousarray(np.asarray(w_fc_sh, np.float32).astype(bfl))
    wprojb = np.ascontiguousarray(np.asarray(w_proj_sh, np.float32).astype(bfl))
    rwT = np.ascontiguousarray(np.asarray(router_w, np.float32).T)
    iotaP = np.arange(128, dtype=np.float32).reshape(128, 1)
    trid = np.triu(np.ones((128, 128), np.float32), 1)
    iota8 = np.arange(E, dtype=np.float32).reshape(1, E)

    xf = x.reshape(N_TOK, C)
    in_maps = []
    for i in range(N_CORES):
        xs = xf[i * TLOC:(i + 1) * TLOC]
        xT = np.ascontiguousarray(xs.T)
        in_maps.append({
            "x_T": xT,
            "x_T16": np.ascontiguousarray(xT.astype(bfl)),
            "x_tm": np.ascontiguousarray(xs),
            "rwT": rwT, "w1": w1b, "w2": w2b,
            "wfc": wfcb, "wproj": wprojb,
            "iotaP": iotaP, "trid": trid, "iota8": iota8,
        })
    return in_maps


def kernel(x, w_fc_sh, w_proj_sh, w1, w2, router_w, balance_bias):
    nc = _get_nc()
    in_maps = make_in_maps(x, w_fc_sh, w_proj_sh, w1, w2, router_w)
    res = run_bass_kernel_spmd(nc, in_maps, list(range(N_CORES)))
    shards = [np.asarray(res.results[i]["o_out"])[:TLOC]
              for i in range(N_CORES)]
    out = np.concatenate(shards, axis=0).reshape(B, T, C).astype(np.float32)
    kernel._last_results = res
    return out


# revision 9
# speedup vs baseline: 1.2921x; 1.0804x over previous
"""MoE layer (shared expert + 8 routed experts, top-2 sigmoid router) on 8
Trainium2 NeuronCores — sparse dispatch version.

Strategy: data-parallel over tokens (1024/core). Each core:
  1. Router in exact fp32 on PE (top-2 via DVE max8 + match_replace).
  2. Sparse dispatch: each token goes to only its top-2 experts via SWDGE
     dma_gather into a per-expert capacity buffer (CAP=384, E[count]=256).
     The gate weight tw is folded in as x*sqrt(tw) before dispatch since
     relu(a*x@w1)^2 @ w2 = a^2 * (relu(x@w1)^2 @ w2).
  3. The slot->token map is built ON DEVICE: exclusive cumsum over the
     top-2 masks (triangular-matrix matmuls), inverted by scatter-adding
     per-token metadata rows into a slot-indexed DRAM array, read back
     directly in the SWDGE index layout via a strided DMA.
  4. Routed experts run in bf16; layer 2 is slot-major (stationary = h^T)
     so outputs are slot rows, scatter-added onto the token-major output
     pre-filled with the shared-expert MLP.

Emission order is tuned so the dispatch-index critical path (DVE + gpsimd)
completes while the PE runs the shared expert, and small index DMAs are
spread across the scalar/vector/gpsimd queues to avoid head-of-line
blocking behind bulk weight DMAs on the sync queue.
"""
import sys

sys.path.insert(0, '/opt/trn_rl_repo')

import numpy as np
import ml_dtypes

import concourse.bass as bass
import concourse.mybir as mybir
import concourse.tile as tile
from concourse import bacc
from concourse.bass_utils import run_bass_kernel_spmd

f32 = mybir.dt.float32
bf16 = mybir.dt.bfloat16
i16 = mybir.dt.int16
AF = mybir.ActivationFunctionType
ALU = mybir.AluOpType

N_CORES = 8
B, T, C = 4, 2048, 768
E, K = 8, 2
N_TOK = B * T
TLOC = N_TOK // N_CORES      # tokens per core (1024)
KT = C // 128                # 6 contraction tiles
TB = TLOC // 128             # 8 token blocks
CAP = 384                    # per-expert slot capacity (mean count = 256)
S = E * CAP                  # 3072 total slots
SB = S // 128                # 24 slot blocks
ECOLS = CAP // 16            # idx columns per expert (24)
EB = CAP // 128              # slot blocks per expert (3)
XS_ROWS = 2 * TLOC + 128     # dispatch buffer rows (rows 2048.. = zeros)
ZROW = 2 * TLOC              # zero row index
OUT_ROWS = TLOC + 128        # output rows (row 1024 = pad-slot dummy)
DUMMY = TLOC


def _build():
    nc = bacc.Bacc("TRN2", target_bir_lowering=False, debug=False,
                   num_devices=N_CORES)

    x_T = nc.declare_dram_parameter("x_T", [C, TLOC], f32, isOutput=False)
    x_T16 = nc.declare_dram_parameter("x_T16", [C, TLOC], bf16, isOutput=False)
    x_tm = nc.declare_dram_parameter("x_tm", [TLOC, C], f32, isOutput=False)
    rwT = nc.declare_dram_parameter("rwT", [C, E], f32, isOutput=False)
    w1 = nc.declare_dram_parameter("w1", [E, C, C], bf16, isOutput=False)
    w2 = nc.declare_dram_parameter("w2", [E, C, C], bf16, isOutput=False)
    wfc = nc.declare_dram_parameter("wfc", [C, C], bf16, isOutput=False)
    wproj = nc.declare_dram_parameter("wproj", [C, C], bf16, isOutput=False)
    iotaP = nc.declare_dram_parameter("iotaP", [128, 1], f32, isOutput=False)
    trid = nc.declare_dram_parameter("trid", [128, 128], f32, isOutput=False)
    iota8 = nc.declare_dram_parameter("iota8", [1, E], f32, isOutput=False)
    o_out = nc.declare_dram_parameter("o_out", [OUT_ROWS, C], f32,
                                      isOutput=True)
    o_dbg = nc.declare_dram_parameter("o_dbg", [128, 16], f32, isOutput=True)

    xs_dram = nc.dram_tensor("xs_dram", [XS_ROWS, C], bf16)
    meta_dram = nc.dram_tensor("meta_dram", [S, 64], f32)

    with tile.TileContext(nc) as tc:
        with (
            tc.tile_pool(name="const", bufs=1) as cpool,
            tc.tile_pool(name="acts", bufs=1) as apool,
            tc.tile_pool(name="wts", bufs=3) as wpool,
            tc.tile_pool(name="rt", bufs=2) as rpool,
            tc.tile_pool(name="masks", bufs=1) as mpool,
            tc.tile_pool(name="idx", bufs=1) as ipool,
            tc.tile_pool(name="xg", bufs=2) as gpool,
            tc.tile_pool(name="hsq", bufs=2) as hpool,
            tc.tile_pool(name="ysb", bufs=2) as ypool,
            tc.tile_pool(name="ysh", bufs=2) as yshpool,
            tc.tile_pool(name="xsc", bufs=2) as xspool,
            tc.tile_pool(name="ps_small", bufs=2, space="PSUM") as ps_s,
            tc.tile_pool(name="ps_l1", bufs=2, space="PSUM") as ps_l1,
            tc.tile_pool(name="ps_l2", bufs=2, space="PSUM") as ps_l2,
        ):
            # ---------------- constants / inputs ----------------
            tri_sb = cpool.tile([128, 128], f32)
            nc.sync.dma_start(tri_sb[:], trid[:])
            iotaP_sb = cpool.tile([128, 1], f32)
            nc.sync.dma_start(iotaP_sb[:], iotaP[:])
            e384_sb = cpool.tile([128, E], f32)
            nc.sync.dma_start(e384_sb[:], iota8[0:1, :].to_broadcast([128, E]))
            nc.vector.tensor_scalar_mul(e384_sb[:], e384_sb[:], float(CAP))
            rwt = cpool.tile([128, KT, E], f32)
            nc.sync.dma_start(rwt[:], rwT.rearrange("(k p) e -> p k e", p=128))
            ones_sb = cpool.tile([128, 128], f32)
            nc.vector.memset(ones_sb[:], 1.0)

            xt = []
            for k in range(KT):
                xt_k = apool.tile([128, TLOC], f32, tag=f"xt{k}")
                nc.sync.dma_start(xt_k[:], x_T[k * 128:(k + 1) * 128, :])
                xt.append(xt_k)

            # zero the slot-metadata array early (gpsimd queue)
            zmeta = ipool.tile([128, SB, 64], f32)
            nc.vector.memset(zmeta[:], 0.0)
            nc.gpsimd.dma_start(meta_dram.rearrange("(b p) c -> p b c", p=128),
                                zmeta[:])

            x16 = apool.tile([128, KT, TLOC], bf16)
            nc.sync.dma_start(x16[:], x_T16.rearrange("(k p) n -> p k n", p=128))
            xtm = apool.tile([128, TB, C], f32)
            nc.sync.dma_start(xtm[:], x_tm.rearrange("(b p) c -> p b c", p=128))

            wfc_sb = wpool.tile([128, KT, C], bf16, tag="w1")
            wproj_sb = wpool.tile([128, KT, C], bf16, tag="w2")
            for k in range(KT):
                nc.sync.dma_start(
                    wfc_sb[:, k, :],
                    wfc.rearrange("(k p) m -> p k m", p=128)[:, k, :])
                nc.sync.dma_start(
                    wproj_sb[:, k, :],
                    wproj.rearrange("(k p) m -> p k m", p=128)[:, k, :])

            slots_tm = ipool.tile([128, 2 * TB], f32)     # col b = k*8+tb
            content = ipool.tile([128, 2 * TB, 64], f32)  # scatter payload
            nc.vector.memset(content[:], 0.0)

            # ---------------- router ----------------
            m_list, m1_list, m2_list = [], [], []
            sqtw_list = []
            for tb in range(TB):
                blk = slice(tb * 128, (tb + 1) * 128)
                ps_r = ps_s.tile([128, E], f32, tag="small")
                for k in range(KT):
                    nc.tensor.matmul(ps_r[:], xt[k][:, blk], rwt[:, k, :],
                                     start=(k == 0), stop=(k == KT - 1))
                scores = rpool.tile([128, E], f32, tag="scores")
                nc.scalar.activation(scores[:], ps_r[:], AF.Sigmoid)
                top8 = rpool.tile([128, E], f32, tag="top8")
                nc.vector.max(top8[:], scores[:])
                mr = rpool.tile([128, E], f32, tag="mr")
                nc.vector.tensor_copy(mr[:, 0:K], top8[:, 0:K])
                nc.vector.memset(mr[:, K:], 0.0)
                zap = rpool.tile([128, E], f32, tag="zap")
                nc.vector.match_replace(zap[:], mr[:], scores[:], 0.0)
                msk = rpool.tile([128, E], f32, tag="msk")
                nc.vector.tensor_sub(msk[:], scores[:], zap[:])
                den = rpool.tile([128, 1], f32, tag="den")
                nc.vector.reduce_sum(den[:], msk[:], mybir.AxisListType.X)
                rden = rpool.tile([128, 1], f32, tag="rden")
                nc.vector.reciprocal(rden[:], den[:])
                twv = rpool.tile([128, K], f32, tag="twv")
                nc.vector.tensor_scalar_mul(twv[:], top8[:, 0:K], rden[:])
                sqtw = mpool.tile([128, K], f32, tag=f"sq{tb}")
                nc.scalar.activation(sqtw[:], twv[:], AF.Sqrt)
                sqtw_list.append(sqtw)

                m_t = mpool.tile([128, E], f32, tag=f"m{tb}")
                nc.vector.tensor_scalar(m_t[:], msk[:], 0.0, None, op0=ALU.is_gt)
                m1_t = mpool.tile([128, E], f32, tag=f"m1{tb}")
                nc.vector.tensor_scalar(m1_t[:], msk[:], top8[:, 0:1], None,
                                        op0=ALU.is_equal)
                m2_t = mpool.tile([128, E], f32, tag=f"m2{tb}")
                nc.vector.tensor_sub(m2_t[:], m_t[:], m1_t[:])
                m_list.append(m_t)
                m1_list.append(m1_t)
                m2_list.append(m2_t)

            # ---------------- positions via exclusive cumsum ----------------
            for tb in range(TB):
                ps_c = ps_s.tile([128, E], f32, tag="small")
                for tb2 in range(tb):
                    nc.tensor.matmul(ps_c[:], ones_sb[:], m_list[tb2][:],
                                     start=(tb2 == 0), stop=False)
                nc.tensor.matmul(ps_c[:], tri_sb[:], m_list[tb][:],
                                 start=(tb == 0), stop=True)
                posc = rpool.tile([128, E], f32, tag="posc")
                nc.vector.tensor_scalar_min(posc[:], ps_c[:], float(CAP - 1))
                sl = rpool.tile([128, E], f32, tag="sl")
                nc.vector.tensor_add(sl[:], posc[:], e384_sb[:])
                t1 = rpool.tile([128, E], f32, tag="t1")
                nc.vector.tensor_mul(t1[:], sl[:], m1_list[tb][:])
                nc.vector.reduce_sum(slots_tm[:, tb:tb + 1], t1[:],
                                     mybir.AxisListType.X)
                t2 = rpool.tile([128, E], f32, tag="t2")
                nc.vector.tensor_mul(t2[:], sl[:], m2_list[tb][:])
                nc.vector.reduce_sum(slots_tm[:, TB + tb:TB + tb + 1], t2[:],
                                     mybir.AxisListType.X)
                # payload: col0 = gather row - 2048, col1 = scatter row - 1024
                nc.vector.tensor_scalar_add(content[:, tb, 0:1], iotaP_sb[:],
                                            float(tb * 128 - 2 * TLOC))
                nc.vector.tensor_scalar_add(content[:, TB + tb, 0:1],
                                            iotaP_sb[:],
                                            float(TLOC + tb * 128 - 2 * TLOC))
                nc.vector.tensor_scalar_add(content[:, tb, 1:2], iotaP_sb[:],
                                            float(tb * 128 - TLOC))
                nc.vector.tensor_scalar_add(content[:, TB + tb, 1:2],
                                            iotaP_sb[:],
                                            float(tb * 128 - TLOC))

            # ---------------- invert the slot map via scatter-add ----------
            slots_i16 = ipool.tile([128, 2 * TB], i16)
            nc.vector.tensor_copy(slots_i16[:], slots_tm[:])
            # scaled dispatch copies (DVE; only needs sqtw) — must precede
            # the first gather but not the scatter/readback chain
            for tb in range(TB):
                for k in range(K):
                    xsk = xspool.tile([128, C], bf16, tag=f"xs{k}")
                    nc.vector.tensor_scalar_mul(xsk[:], xtm[:, tb, :],
                                                sqtw_list[tb][:, k:k + 1])
                    nc.sync.dma_start(
                        xs_dram[k * TLOC + tb * 128:k * TLOC + (tb + 1) * 128, :],
                        xsk[:])
            zrow16 = ipool.tile([128, C], bf16)
            nc.vector.memset(zrow16[:], 0.0)
            nc.sync.dma_start(xs_dram[ZROW:ZROW + 128, :], zrow16[:])
            nc.sync.dma_start(o_dbg[:], slots_tm[:])
            inv_idx = ipool.tile([128, 2 * TLOC // 16], i16)  # [128, 128]
            inv_r = inv_idx[:].rearrange("p (c r) -> p c r", r=8)
            for r in range(8):
                eng = nc.scalar if r % 2 == 0 else nc.gpsimd
                eng.dma_start(inv_r[0:16, :, r],
                              slots_i16[r * 16:(r + 1) * 16, :])
            for rr in range(1, 8):
                eng = nc.scalar if rr % 2 == 0 else nc.gpsimd
                eng.dma_start(inv_idx[rr * 16:(rr + 1) * 16, :],
                              inv_idx[0:16, :])

            nc.gpsimd.dma_scatter_add(meta_dram[:], content[:], inv_idx[:],
                                      2 * TLOC, 2 * TLOC, 64)
            # strided readback straight into the SWDGE idx wrap layout:
            # row (b*128 + r*16 + q) -> [q, b*8+r]
            gsb = ipool.tile([16, S // 16, 2], f32)
            meta_r = meta_dram.rearrange("(b r q) c -> q (b r) c", q=16, r=8)
            nc.gpsimd.dma_start(gsb[:], meta_r[:, :, 0:2])

            gidx_f = ipool.tile([16, S // 16], f32)
            nc.vector.tensor_scalar(gidx_f[:], gsb[:, :, 0], float(ZROW),
                                    float(ZROW), op0=ALU.add, op1=ALU.min)
            nc.vector.tensor_scalar_max(gidx_f[:], gidx_f[:], 0.0)
            sidx_f = ipool.tile([16, S // 16], f32)
            nc.vector.tensor_scalar(sidx_f[:], gsb[:, :, 1], float(DUMMY),
                                    float(DUMMY), op0=ALU.add, op1=ALU.min)
            nc.vector.tensor_scalar_max(sidx_f[:], sidx_f[:], 0.0)

            gidx16 = ipool.tile([128, S // 16], i16)
            sidx16 = ipool.tile([128, S // 16], i16)
            nc.vector.tensor_copy(gidx16[0:16, :], gidx_f[:])
            nc.vector.tensor_copy(sidx16[0:16, :], sidx_f[:])
            for rr in range(1, 8):
                nc.scalar.dma_start(gidx16[rr * 16:(rr + 1) * 16, :],
                                    gidx16[0:16, :])
                nc.gpsimd.dma_start(sidx16[rr * 16:(rr + 1) * 16, :],
                                    sidx16[0:16, :])

            # ---------------- shared expert (bf16) ----------------
            h_sh = apool.tile([128, KT, TLOC], bf16)
            for ho in range(KT):
                mo = slice(ho * 128, (ho + 1) * 128)
                for th in range(2):
                    ts = slice(th * 512, (th + 1) * 512)
                    ps = ps_l1.tile([128, 512], f32, tag="l1")
                    for k in range(KT):
                        nc.tensor.matmul(ps[:], wfc_sb[:, k, mo],
                                         x16[:, k, ts],
                                         start=(k == 0), stop=(k == KT - 1))
                    rl = yshpool.tile([128, 512], f32, tag="rl")
                    nc.vector.tensor_scalar_max(rl[:], ps[:], 0.0)
                    nc.scalar.activation(h_sh[:, ho, ts], rl[:], AF.Square)
            for tcb in range(TB):
                tcs = slice(tcb * 128, (tcb + 1) * 128)
                ysh_t = yshpool.tile([128, C], f32, tag="ysh")
                for half in range(2):
                    hs = slice(half * 384, (half + 1) * 384)
                    psx = ps_l2.tile([128, 384], f32, tag=f"l2{half}")
                    for hk in range(KT):
                        nc.tensor.matmul(psx[:], h_sh[:, hk, tcs],
                                         wproj_sb[:, hk, hs],
                                         start=(hk == 0), stop=(hk == KT - 1))
                    nc.scalar.activation(ysh_t[:, hs], psx[:], AF.Copy)
                nc.sync.dma_start(o_out[tcb * 128:(tcb + 1) * 128, :], ysh_t[:])
            zrow32 = ipool.tile([128, C], f32)
            nc.vector.memset(zrow32[:], 0.0)
            nc.sync.dma_start(o_out[DUMMY:DUMMY + 128, :], zrow32[:])

            # ---------------- routed experts ----------------
            def emit_gather(e):
                xg = gpool.tile([128, KT, CAP], bf16, tag="xg")
                nc.gpsimd.dma_gather(xg[:], xs_dram[:],
                                     gidx16[:, ECOLS * e:ECOLS * (e + 1)],
                                     CAP, CAP, C, transpose=True)
                return xg

            xg_cur = emit_gather(0)
            for e in range(E):
                w1sb = wpool.tile([128, KT, C], bf16, tag="w1")
                w2sb = wpool.tile([128, KT, C], bf16, tag="w2")
                w1_src = w1[e].rearrange("(k p) m -> p k m", p=128)
                w2_src = w2[e].rearrange("(k p) m -> p k m", p=128)
                for k in range(KT):
                    nc.sync.dma_start(w1sb[:, k, :], w1_src[:, k, :])
                    nc.sync.dma_start(w2sb[:, k, :], w2_src[:, k, :])

                xg_next = emit_gather(e + 1) if e + 1 < E else None

                hsq = hpool.tile([128, KT, CAP], bf16, tag="hsq")
                for ho in range(KT):
                    mo = slice(ho * 128, (ho + 1) * 128)
                    ps = ps_l1.tile([128, 512], f32, tag="l1")
                    for k in range(KT):
                        nc.tensor.matmul(ps[:, 0:CAP], w1sb[:, k, mo],
                                         xg_cur[:, k, :],
                                         start=(k == 0), stop=(k == KT - 1))
                    rl = yshpool.tile([128, 512], f32, tag="rl")
                    nc.vector.tensor_scalar_max(rl[:, 0:CAP], ps[:, 0:CAP], 0.0)
                    nc.scalar.activation(hsq[:, ho, :], rl[:, 0:CAP], AF.Square)

                ysb = ypool.tile([128, EB, C], f32, tag="ysb")
                for sc in range(EB):
                    scs = slice(sc * 128, (sc + 1) * 128)
                    for half in range(2):
                        hs = slice(half * 384, (half + 1) * 384)
                        psx = ps_l2.tile([128, 384], f32, tag=f"l2{half}")
                        for hk in range(KT):
                            nc.tensor.matmul(psx[:], hsq[:, hk, scs],
                                             w2sb[:, hk, hs],
                                             start=(hk == 0),
                                             stop=(hk == KT - 1))
                        nc.scalar.activation(ysb[:, sc, hs], psx[:], AF.Copy)
                nc.gpsimd.dma_scatter_add(o_out[:], ysb[:],
                                          sidx16[:, ECOLS * e:ECOLS * (e + 1)],
                                          CAP, CAP, C)
                xg_cur = xg_next
    nc.compile()
    return nc


_NC_CACHE = None


def _get_nc():
    global _NC_CACHE
    if _NC_CACHE is None:
        _NC_CACHE = _build()
    return _NC_CACHE


def make_in_maps(x, w_fc_sh, w_proj_sh, w1, w2, router_w):
    x = np.ascontiguousarray(np.asarray(x, np.float32))
    bfl = ml_dtypes.bfloat16
    w1b = np.ascontiguousarray(np.asarray(w1, np.float32).astype(bfl))
    w2b = np.ascontiguousarray(np.asarray(w2, np.float32).astype(bfl))
    wfcb = np.ascontiguousarray(np.asarray(w_fc_sh, np.float32).astype(bfl))
    wprojb = np.ascontiguousarray(np.asarray(w_proj_sh, np.float32).astype(bfl))
    rwT = np.ascontiguousarray(np.asarray(router_w, np.float32).T)
    iotaP = np.arange(128, dtype=np.float32).reshape(128, 1)
    trid = np.triu(np.ones((128, 128), np.float32), 1)
    iota8 = np.arange(E, dtype=np.float32).reshape(1, E)

    xf = x.reshape(N_TOK, C)
    in_maps = []
    for i in range(N_CORES):
        xs = xf[i * TLOC:(i + 1) * TLOC]
        xT = np.ascontiguousarray(xs.T)
        in_maps.append({
            "x_T": xT,
            "x_T16": np.ascontiguousarray(xT.astype(bfl)),
            "x_tm": np.ascontiguousarray(xs),
            "rwT": rwT, "w1": w1b, "w2": w2b,
            "wfc": wfcb, "wproj": wprojb,
            "iotaP": iotaP, "trid": trid, "iota8": iota8,
        })
    return in_maps


def kernel(x, w_fc_sh, w_proj_sh, w1, w2, router_w, balance_bias):
    nc = _get_nc()
    in_maps = make_in_maps(x, w_fc_sh, w_proj_sh, w1, w2, router_w)
    res = run_bass_kernel_spmd(nc, in_maps, list(range(N_CORES)))
    shards = [np.asarray(res.results[i]["o_out"])[:TLOC]
              for i in range(N_CORES)]
    out = np.concatenate(shards, axis=0).reshape(B, T, C).astype(np.float32)
    kernel._last_results = res
    return out
